# revision 3
# baseline (speedup 1.0000x reference)
"""Trainium2 Bass kernel for nn_MultiHeadAttention_84576495993495.

Math: the reference module's output einsum is
    out = einsum('bhqk,bhvo->bhvo', attn, v)
which contracts softmax(attn) over BOTH q and k. Every softmax row sums
to 1, so sum_{q,k} attn == S (= 2048) and the whole attention block
collapses to out == S * v. Hence

    reference(x, ...) == ((x @ Wv.T + bv) * S) @ Wp.T + bp == x @ M + c
with
    M = S * Wv.T @ Wp.T          (folded on host in fp64)
    c = S * Wp @ bv + bp

(verified vs the jax reference: rel Frobenius err ~3.6e-7 = fp32 noise).

Device work: the data-dependent GEMM y = x @ M + c, sharded data-parallel
over the 8192 rows -> 1024 rows per NeuronCore, in ONE bf16 pass
(rel err 2.0e-3 on HW vs the 2e-2 harness tolerance).

Schedule (v2, this session; HW-measured on 8 cores via a device-resident
persistent-jit For_i-slope bench with ~1ms wall noise):

- TRANSPOSED orientation: M-tiles are the stationary operand, x streams;
  PSUM banks hold y^T tiles [128 d_out x 512 rows]. The bias c[d_out] is
  then a per-partition scalar, so tails use tensor_scalar_add on DVE and
  activation-Identity-with-bias on ACT -- two engines drain PSUM in
  parallel and no [128, D] bias table is streamed.
- 128 LDW+MM pairs of N=512 bf16 per core-iteration. Measured floor
  272-275 ns/MM = 512-cycle stream @2.4GHz + ~53ns serialized FWL
  weight load (not hidable: bass emits per-MM InstLdweights; walrus's
  --enable-ldw-opt rejects them, same-weights-adjacent ordering measures
  equal, N>512 out trips the s3d3_mm_num_elements ISA cap, and a
  row-group-disjoint K-split -- K=96 @ (0,0) + K=32 @ tile_position
  (96,0) per stationary tile, hoping the group-3 LDW pulls ahead of the
  in-flight group-0-2 MM -- accumulates CORRECTLY into the shared bank
  but measures 3.9x SLOWER (1198 vs 306 ns per k-tile-bank): partial-
  array MMs at offset positions serialize with heavy per-MM overhead).
- Groups of 4 fixed PSUM banks (2 d_out-tiles x 2 row-chunks), two bank
  sets alternating; output staged in a partition-major [128, 8, 1024]
  fp16 mega tile; host reassembles the transpose and upcasts.
- Steady-state loop build ("loop2"): DEFERRED DRAIN -- each phase first
  finishes the PREVIOUS phase's last-group tails, then stores the
  previous mega tile, then runs its own 4 groups. The For_i back-edge
  never waits on tails or stores, so the loop slope equals the pure-GEMM
  floor: measured 34983-35156 ns/iter vs 35052-35156 for GEMM-only
  (inline-store variant: 40407; baseline kernel: 41207 same session).
- Single-shot build: inline per-group tails+stores, last nt-pair split
  across both HWDGE rings to minimize the end-of-kernel drain; input
  k-slices split across both rings so the PE is never load-gated past
  the first slice.
"""

import os
from functools import lru_cache

import numpy as np

# Defensive: a previous run crashing mid-execution can leave the NeuronCores
# in an unrecoverable state (NRT_EXEC_UNIT_UNRECOVERABLE); resetting cores at
# NRT init clears it and is harmless otherwise.
os.environ.setdefault("NEURON_RT_RESET_CORES", "1")

import concourse.bass as bass
import concourse.mybir as mybir
import concourse.tile as tile
from concourse import bacc
from concourse.bass_utils import run_bass_kernel_spmd

N_CORES = 8
P = 128
D = 1024                      # model dim (= SLICE_SIZE)
B, S = 4, 2048
R_TOTAL = B * S               # 8192 rows
R_CORE = R_TOTAL // N_CORES   # 1024 rows per core
K_TILES = D // P              # 8 contraction tiles
NT = D // P                   # 8 d_out tiles
RCH = 2                       # row chunks of 512
SCALE = float(S)              # sum over (q,k) of softmax rows == S


@lru_cache(maxsize=8)
def _build_nc(loop_iters: int | None = None, unroll: int = 1):
    """loop_iters None -> single-shot kernel (drain-minimized).
    loop_iters set -> steady-state bench build: tc.For_i hardware loop with
    the deferred-drain two-phase schedule; `unroll` phase-pairs per
    back-edge."""
    nc = bacc.Bacc(None, target_bir_lowering=False)

    x = nc.dram_tensor("x", [D, R_CORE], mybir.dt.bfloat16, kind="ExternalInput")
    m = nc.dram_tensor("m", [D, D], mybir.dt.bfloat16, kind="ExternalInput")
    cb = nc.dram_tensor("cb", [P, NT], mybir.dt.float32, kind="ExternalInput")
    y = nc.dram_tensor("y", [P, NT, R_CORE], mybir.dt.float16, kind="ExternalOutput")

    x_t = x.rearrange("(ko p) r -> p ko r", p=P)   # [128, 8, 1024]
    m_t = m.rearrange("(ko p) n -> p ko n", p=P)   # [128, 8, 1024]

    with tile.TileContext(nc) as tc:
        with (
            tc.tile_pool(name="wpool", bufs=1) as wpool,
            tc.tile_pool(name="opool", bufs=2) as opool,
            tc.tile_pool(name="pspool", bufs=8, space="PSUM") as pspool,
        ):
            x_sb = wpool.tile([P, K_TILES, R_CORE], mybir.dt.bfloat16, tag="x_sb")
            m_sb = wpool.tile([P, K_TILES, D], mybir.dt.bfloat16, tag="m_sb")
            cb_sb = wpool.tile([P, NT], mybir.dt.float32, tag="cb_sb")

            nc.scalar.dma_start(cb_sb[:], cb[:])
            # Split every k-slice across both HWDGE rings so each slice
            # lands in ~half the time -- the PE's k-major accumulation in
            # group 0 is then never DMA-gated past the first slice.
            for k in range(K_TILES):
                nc.sync.dma_start(m_sb[:, k, 0:512], m_t[:, k, 0:512])
                nc.scalar.dma_start(m_sb[:, k, 512:D], m_t[:, k, 512:D])
                nc.sync.dma_start(x_sb[:, k, 0:512], x_t[:, k, 0:512])
                nc.scalar.dma_start(x_sb[:, k, 512:R_CORE], x_t[:, k, 512:R_CORE])

            def tail(bank, nt, rch, dest):
                # y^T tile + per-partition bias scalar; DVE for rch 0,
                # ACT for rch 1 so the two tails of a d_out tile run on
                # different engines.
                if rch == 0:
                    nc.vector.tensor_scalar_add(
                        dest[:, nt, bass.ts(rch, 512)],
                        bank[:],
                        cb_sb[:, nt : nt + 1],
                    )
                else:
                    nc.scalar.activation(
                        dest[:, nt, bass.ts(rch, 512)],
                        bank[:],
                        mybir.ActivationFunctionType.Identity,
                        bias=cb_sb[:, nt : nt + 1],
                    )

            def group_mms(ps, g):
                # 32 matmuls: k-major accumulation, the two row-chunk MMs
                # of each stationary tile back-to-back.
                for k in range(K_TILES):
                    for j in range(2):
                        nt = 2 * g + j
                        for rch in range(RCH):
                            nc.tensor.matmul(
                                ps[j * 2 + rch][:],
                                m_sb[:, k, bass.ts(nt, P)],
                                x_sb[:, k, bass.ts(rch, 512)],
                                start=(k == 0),
                                stop=(k == K_TILES - 1),
                            )

            if loop_iters is None:
                # ---- single-shot: inline tails + stores ----
                mega = opool.tile(
                    [P, NT, R_CORE], mybir.dt.float16, tag="mega", name="mega"
                )
                for g in range(4):
                    ps = [
                        pspool.tile([P, 512], mybir.dt.float32, tag="ps", name="ps")
                        for _ in range(4)
                    ]
                    group_mms(ps, g)
                    for j in range(2):
                        nt = 2 * g + j
                        for rch in range(RCH):
                            tail(ps[j * 2 + rch], nt, rch, mega)
                    if g < 3:
                        eng = nc.sync if g % 2 == 0 else nc.scalar
                        eng.dma_start(
                            y[:, 2 * g : 2 * g + 2], mega[:, 2 * g : 2 * g + 2]
                        )
                    else:
                        nc.sync.dma_start(y[:, 6:7], mega[:, 6:7])
                        nc.scalar.dma_start(y[:, 7:8], mega[:, 7:8])
            else:
                # ---- steady-state loop: deferred drain ----
                mg = [
                    wpool.tile(
                        [P, NT, R_CORE], mybir.dt.float16, tag=f"mg{i}", name=f"mg{i}"
                    )
                    for i in range(2)
                ]
                ps8 = [
                    pspool.tile([P, 512], mybir.dt.float32, tag="ps", name=f"ps8_{i}")
                    for i in range(8)
                ]

                def phase(p):
                    cur, prev = mg[p], mg[1 - p]
                    # (1) finish prev phase's g3 tails (banks 4:8)
                    for j in range(2):
                        for rch in range(RCH):
                            tail(ps8[4 + j * 2 + rch], 6 + j, rch, prev)
                    # (2) store prev mega (hidden under this phase's GEMM)
                    nc.sync.dma_start(y[:, 0:4], prev[:, 0:4])
                    nc.scalar.dma_start(y[:, 4:8], prev[:, 4:8])
                    # (3) compute: 4 groups on alternating bank halves
                    for g in range(4):
                        bank0 = (g % 2) * 4
                        group_mms(ps8[bank0 : bank0 + 4], g)
                        if g < 3:
                            for j in range(2):
                                nt = 2 * g + j
                                for rch in range(RCH):
                                    tail(ps8[bank0 + j * 2 + rch], nt, rch, cur)

                phases_per_iter = 2 * max(1, unroll)
                with tc.For_i(0, max(1, loop_iters // phases_per_iter), 1):
                    for _ in range(max(1, unroll)):
                        phase(0)
                        phase(1)
                # epilogue: finish the final phase's g3 tails + store mg[1]
                for j in range(2):
                    for rch in range(RCH):
                        tail(ps8[4 + j * 2 + rch], 6 + j, rch, mg[1])
                nc.sync.dma_start(y[:, 0:4], mg[1][:, 0:4])
                nc.scalar.dma_start(y[:, 4:8], mg[1][:, 4:8])
    nc.compile()
    return nc


def _host_prep(x, Wv, bv, Wp, bp):
    import ml_dtypes

    X = np.ascontiguousarray(x, dtype=np.float32).reshape(R_TOTAL, D)
    M64 = SCALE * (Wv.T.astype(np.float64) @ Wp.T.astype(np.float64))
    c = (SCALE * (Wp.astype(np.float64) @ bv.astype(np.float64)) + bp).astype(
        np.float32
    )
    cbt = np.ascontiguousarray(c.reshape(NT, P).T)  # [P, NT]
    Mb = M64.astype(ml_dtypes.bfloat16)

    in_maps = []
    for i in range(N_CORES):
        shard_t = np.ascontiguousarray(X[i * R_CORE : (i + 1) * R_CORE].T).astype(
            ml_dtypes.bfloat16
        )
        in_maps.append({"x": shard_t, "m": Mb, "cb": cbt})
    return in_maps


def _fix(a):
    # [P, NT, R] fp16 (y^T partition-major) -> [R, D] fp32
    a = np.asarray(a)
    return a.transpose(2, 1, 0).reshape(R_CORE, D).astype(np.float32)


def kernel(x, Wq, bq, Wk, bk, Wv, bv, Wp, bp):
    x, Wv, bv, Wp, bp = (np.asarray(a) for a in (x, Wv, bv, Wp, bp))
    nc = _build_nc(None)
    in_maps = _host_prep(x, Wv, bv, Wp, bp)
    res = run_bass_kernel_spmd(nc, in_maps, core_ids=list(range(N_CORES)))
    ys = [_fix(r["y"]) for r in res.results]
    return np.concatenate(ys, axis=0).reshape(B, S, D)
